# revision 30
# baseline (speedup 1.0000x reference)
"""Grouped 2-layer MLP (ConvNN) Trainium2 kernel — 4-quadrant PE tiling.

Math (per group g of SIZE=2048):
    h[b,g,:]   = LeakyReLU_0.2(W0[g] @ x[b] + b0[g])     (64 -> 64)
    out[b,g,:] = W1[g] @ h[b,g,:] + b1[g]                (64 -> 64)

Strategy:
  - Shard the group axis over 8 cores (256 groups/core = 128 pairs),
    fully independent, no collectives.
  - Both layers contract over K=64, so the 128x128 PE array is split
    into four independent 64x64 tiles (tile_position mode 64x64:
    T0=(0,0), T2=(0,64), T8=(64,0), T10=(64,64)).  Per slot t all four
    quadrants stream CONCURRENTLY on pair t (layer 0: T8+T2) and pair
    t-LAG (layer 1: T0+T10), each quadrant covering B=1024 in two
    N=512 matmuls with CROSSED banks so the two concurrent streams of
    a layer never sustain writes to the same PSUM bank:
      phase 0: T8 -> hb0[0:64],  T2 -> hb1[64:128]   (and T0/T10 on out)
      phase 1: T8 -> hb1[0:64],  T2 -> hb0[64:128]
    Phase interleaving also keeps a quadrant's two matmuls nonadjacent
    in pc order — matmul STARTS are pc-monotone, so adjacent same-
    quadrant pairs would cascade every later start (measured 1.95us
    slots instead of ~1.24us).  Completing each pair's [128,1024] psum
    tile WITHIN its own slot (not staggered across two) is what lets
    the 2-buffer psum rotation run at P ~= 1.24us: write(slot s) ->
    evac(s..s+1) -> reuse(s+2).
  - x is duplicated on both partition halves so the two layer-0
    quadrants (disjoint SBUF partition ranges) share the xbus without
    conflict.  Stationaries are compact [64,64] slices — no block-
    diagonal zero padding — and each quadrant's second matmul reuses
    the loaded weights (ldweights=False; per-quadrant weight state).
  - Evacuation is the bottleneck: only ACT/DVE can read PSUM, fp32
    psum caps DVE at 1x.  ACT Prelu(+b0, fp16 out) drains h (1.11us
    per [128,1024]); DVE tensor_scalar(+b1, fp16 out) drains out
    (1.285us dur, 1.192us issue rate = (120+1024)/0.96 exactly).  The
    binding loop is obank-reuse: T0(p) waits DVE(p-2); in-slot 1.07us
    to the last out-write + 1.285us DVE -> P ~= 1.2-1.25us.  Splitting
    evacs per-bank measured WORSE (fixed ~220-260ns/op overhead), and
    16-bit PSUM (which would give DVE 2x) is TRN3-only.
  - PE clock: HAM un-throttles to 2.4 GHz ~20us in PROVIDED the
    pipeline never stalls >3.4us (a single mid-kernel stall measured
    re-throttling it to 1.2 GHz for the rest of the run).  Cold
    per-quadrant cycle 854 stream + 203 drain + 125 ldweights ~=
    1.18us just fits under the DVE; warm (~380-400ns/MM) leaves slack,
    so the steady state locks to the DVE at exactly 1192ns/slot (974
    of 1024 MMs measured warm).  Same-bank concurrent PE writes to
    different partition halves are legal (verified correct); only
    sustained same-bank streams are slow.
  - opool bufs=12 is load-bearing: with 5, DVE(p) WARs on store(p-5),
    whose completion is delayed ~8us at startup by bulk-weight DMA
    contention — that one stall cost 4.6us AND re-throttled the PE.
    (bufs=24 regressed 30us — SBUF layout shifts are fragile here;
    so did moving startup loads to the sync ring or merging them.)
  - Output written fp16 as [pair, 128, B] (256KB/pair on the sync
    HWDGE ring; inputs ride gpsimd SWDGE; the per-dma_start ~0.7us Q7
    descriptor-gen cost keeps the startup head to 3 transfers),
    un-transposed/cast to fp32 on the host.

History: full-array baseline 271us (PE-streaming-bound, 512 serial
matmuls) -> 263us (quadrants, naive order) -> 182us (phase
interleave + in-slot psum completion) -> 179us (pool tuning) ->
174us (opool 12 kills the ramp stall; PE stays warm; DVE pegged at
its 1192ns theoretical rate) -> ~173us (LAG=2, 32KB first weight
chunk).  Floor: ~10.5us boot+loads, 128 x 1.192us DVE stream,
~4us tail.  DO NOT perturb the DMA/dependency structure casually:
splitting the x load into 2 transfers, merging head loads into one
tile, sync-ring startup loads, and opool=24 EACH regressed ~30us
(~270ns/slot — consistent with the DVE losing its op-to-op dispatch
overlap when its dependency/sem pattern changes)."""

from contextlib import ExitStack

import numpy as np

import concourse.bass as bass
import concourse.mybir as mybir
import concourse.tile as tile
from concourse.bass_utils import run_bass_kernel_spmd

B = 1024
IN_DIM = 64
SIZE = 2048
D1 = 64
D2 = 64
NEG_SLOPE = 0.2
N_CORES = 8
GPC = SIZE // N_CORES  # 256 groups per core
NPAIR = GPC // 2  # 128 group-pairs per core
LAG = 2  # software-pipeline distance between layer 0 and layer 1

_NC_CACHE = None


def _build():
    global _NC_CACHE
    if _NC_CACHE is not None:
        return _NC_CACHE

    f32 = mybir.dt.float32
    f16 = mybir.dt.float16

    nc = bass.Bass()
    xt = nc.declare_dram_parameter("xt", [128, B], f16, isOutput=False)
    w0c = nc.declare_dram_parameter("w0c", [128, NPAIR, 64], f16, isOutput=False)
    w1c = nc.declare_dram_parameter("w1c", [128, NPAIR, 64], f16, isOutput=False)
    bp = nc.declare_dram_parameter("bp", [128, 2, NPAIR], f32, isOutput=False)
    out = nc.declare_dram_parameter("out", [NPAIR, 128, B], f16, isOutput=True)

    with ExitStack() as ctx:
        tc = ctx.enter_context(tile.TileContext(nc))
        singles = ctx.enter_context(tc.tile_pool(name="singles", bufs=1))
        hpool = ctx.enter_context(tc.tile_pool(name="hpool", bufs=6))
        opool = ctx.enter_context(tc.tile_pool(name="opool", bufs=12))
        pspool = ctx.enter_context(tc.tile_pool(name="psum", bufs=2, space="PSUM"))

        # Startup-critical loads first (each gpsimd dma_start costs
        # ~0.7us of Q7 descriptor-gen, so the head uses as few transfers
        # as possible): xt + w0 head gate T8(0); biases gate ACT(0); w1
        # head gates T0(0) at slot LAG.  Bulk weights stream behind.
        xsb = singles.tile([128, B], f16)
        nc.gpsimd.dma_start(out=xsb, in_=xt[:])
        w0sb = singles.tile([128, NPAIR, 64], f16)
        w1sb = singles.tile([128, NPAIR, 64], f16)
        bsb = singles.tile([128, 2, NPAIR], f32)
        b0sb = bsb[:, 0, :]
        b1sb = bsb[:, 1, :]
        edges = [0, 2, 8, 40, 72, 104, NPAIR]
        nc.gpsimd.dma_start(
            out=w0sb[:, edges[0] : edges[1], :], in_=w0c[:, edges[0] : edges[1], :]
        )
        nc.gpsimd.dma_start(out=bsb, in_=bp[:])
        nc.gpsimd.dma_start(
            out=w1sb[:, edges[0] : edges[1], :], in_=w1c[:, edges[0] : edges[1], :]
        )
        for c in range(1, len(edges) - 1):
            sl = slice(edges[c], edges[c + 1])
            nc.gpsimd.dma_start(out=w0sb[:, sl, :], in_=w0c[:, sl, :])
            nc.gpsimd.dma_start(out=w1sb[:, sl, :], in_=w1c[:, sl, :])

        def w0_at(t):
            return w0sb[:, t, :]

        def w1_at(p):
            return w1sb[:, p, :]

        def quad_mm(psum_tile, prange, lhsT, rhs_tile, rrange, tile_pos, col_half, first):
            """One N=512 matmul (one PSUM bank).  first=False reuses the
            stationary loaded by this quadrant's first matmul of the slot
            (per-tile weight state; other quadrants' LDWEIGHTS don't
            disturb it)."""
            n0, n1 = (0, 512) if col_half == 0 else (512, 1024)
            mm = nc.tensor.matmul(
                psum_tile[prange[0] : prange[1], n0:n1],
                lhsT,
                rhs_tile[rrange[0] : rrange[1], n0:n1],
                start=True,
                stop=True,
                tile_position=tile_pos,
            )
            if not first:
                mm.ins.ldweights = False

        hs_live = {}
        for t in range(NPAIR + LAG):
            p = t - LAG  # pair running layer 1 this slot
            # Both quadrant-pairs of a layer work on the SAME pair within
            # one slot, with CROSSED batch-halves so the two concurrent
            # streams always write different PSUM banks:
            #   phase 0: T8 -> bank0[0:64],  T2 -> bank1[64:128]
            #   phase 1: T8 -> bank1[0:64],  T2 -> bank0[64:128]
            # The pair's full [128,1024] psum tile completes ~1.1us into
            # its own slot, so its evac (issued same slot) frees the
            # 2-buffer rotation a full 2 slots later (P >= ~1.2us), not
            # the ~2.2us the staggered layout forced.  Phases also
            # interleave the quadrants so pc-monotone matmul starts don't
            # cascade (a quadrant's two halves are never adjacent).
            if t < NPAIR:
                hp = pspool.tile([128, B], f32, tag="hps", name=f"hp{t}")
            if 0 <= p < NPAIR:
                op = pspool.tile([128, B], f32, tag="ops", name=f"op{p}")
            for phase in (0, 1):
                first = phase == 0
                if t < NPAIR:  # layer 0 of pair t (T8 + T2)
                    quad_mm(
                        hp, (0, 64), w0_at(t)[64:128, :],
                        xsb, (64, 128), (64, 0), phase, first,
                    )
                    quad_mm(
                        hp, (64, 128), w0_at(t)[0:64, :],
                        xsb, (0, 64), (0, 64), 1 - phase, first,
                    )
                if 0 <= p < NPAIR:  # layer 1 of pair p (T0 + T10)
                    quad_mm(
                        op, (0, 64), w1_at(p)[0:64, :],
                        hs_live[p], (0, 64), (0, 0), phase, first,
                    )
                    quad_mm(
                        op, (64, 128), w1_at(p)[64:128, :],
                        hs_live[p], (64, 128), (64, 64), 1 - phase, first,
                    )
            if t < NPAIR:
                hs = hpool.tile([128, B], f16, tag="h", name=f"hs{t}")
                nc.scalar.activation(
                    out=hs,
                    in_=hp,
                    func=mybir.ActivationFunctionType.Prelu,
                    bias=b0sb[:, t : t + 1],
                    scale=1.0,
                    alpha=NEG_SLOPE,
                )
                hs_live[t] = hs
            if 0 <= p < NPAIR:
                hs_live.pop(p)
                osb = opool.tile([128, B], f16, tag="o", name=f"os{p}")
                nc.vector.tensor_scalar_add(osb, op, b1sb[:, p : p + 1])
                nc.sync.dma_start(out=out[p], in_=osb)

    _dedupe_ldweights(nc)
    _split_multi_waits(nc)
    _NC_CACHE = nc
    return nc


def _dedupe_ldweights(nc):
    """Drop LDWEIGHTS whose AP is identical to the previous one on the PE
    queue (the two half-batch matmuls of a (pair, quadrant) share one
    stationary), migrating semaphore info onto the next PE instruction."""
    import json

    def ldw_key(inst):
        try:
            return mybir.instruction_to_pretty_json_string(inst)
        except Exception:
            return None

    def strip_name(js):
        d = json.loads(js)
        d.pop("name", None)
        d.pop("debug", None)
        d.pop("sync_info", None)
        return json.dumps(d, sort_keys=True)

    n = 0
    for f in nc.m.functions:
        for bb in f.blocks:
            prev_key = None
            out_insts = []
            pending_sync = None
            for inst in bb.instructions:
                eng = getattr(inst, "engine", None)
                if isinstance(inst, mybir.InstLdweights):
                    key = ldw_key(inst)
                    key = strip_name(key) if key else None
                    if key is not None and key == prev_key:
                        si = inst.sync_info
                        if si is not None and (si.on_wait or si.on_update):
                            pending_sync = si
                        n += 1
                        continue  # drop duplicate
                    prev_key = key
                elif eng == mybir.EngineType.PE and pending_sync is not None:
                    si = inst.sync_info
                    waits = list(pending_sync.on_wait or [])
                    upds = list(pending_sync.on_update or [])
                    if si is not None:
                        waits += list(si.on_wait or [])
                        upds += list(si.on_update or [])
                    inst.sync_info = mybir.SyncInfo(on_wait=waits, on_update=upds)
                    pending_sync = None
                out_insts.append(inst)
            assert pending_sync is None, "dangling sync from dropped ldweights"
            bb.instructions = out_insts
    return n


def _split_multi_waits(nc):
    """Walrus allows at most ONE semaphore wait per instruction; hoist
    extras onto same-engine NoOp carriers directly before it."""
    import bass_rust

    n = 0
    for f in nc.m.functions:
        for bb in f.blocks:
            out_insts = []
            changed = False
            for inst in bb.instructions:
                si = inst.sync_info
                waits = list(si.on_wait) if si is not None and si.on_wait else []
                if len(waits) > 1:
                    changed = True
                    for w in waits[:-1]:
                        nop = bass_rust.InstNoOp(
                            name=f"{inst.name}-sw{n}", engine=inst.engine
                        )
                        n += 1
                        nop.sync_info = mybir.SyncInfo(on_wait=[w], on_update=[])
                        out_insts.append(nop)
                    inst.sync_info = mybir.SyncInfo(
                        on_wait=[waits[-1]],
                        on_update=list(si.on_update) if si.on_update else [],
                    )
                out_insts.append(inst)
            if changed:
                bb.instructions = out_insts
    return nc


def _prepare_in_maps(x, W0, b0, W1, b1):
    x = np.asarray(x, dtype=np.float32)
    xt1 = np.ascontiguousarray(x.T).astype(np.float16)  # (64, 1024)
    xt = np.concatenate([xt1, xt1], axis=0)  # (128, 1024) duplicated halves
    in_maps = []
    for c in range(N_CORES):
        sl = slice(c * GPC, (c + 1) * GPC)
        W0c = np.asarray(W0[sl], dtype=np.float32).reshape(NPAIR, 2, D1, IN_DIM)
        W1c = np.asarray(W1[sl], dtype=np.float32).reshape(NPAIR, 2, D2, D1)
        # w0[64+k, p, j] = W0[2p, j, k] (A, tile T8 rows 64-127)
        # w0[k,    p, j] = W0[2p+1, j, k] (B, tile T2 rows 0-63)
        w0ck = np.empty((128, NPAIR, 64), dtype=np.float16)
        w0ck[64:128] = W0c[:, 0].transpose(2, 0, 1)
        w0ck[0:64] = W0c[:, 1].transpose(2, 0, 1)
        # w1[k,    p, j] = W1[2p, j, k] (A, tile T0 rows 0-63)
        # w1[64+k, p, j] = W1[2p+1, j, k] (B, tile T10 rows 64-127)
        w1ck = np.empty((128, NPAIR, 64), dtype=np.float16)
        w1ck[0:64] = W1c[:, 0].transpose(2, 0, 1)
        w1ck[64:128] = W1c[:, 1].transpose(2, 0, 1)

        bpc = np.stack(
            [
                np.asarray(b0[sl], dtype=np.float32).reshape(NPAIR, 128).T,
                np.asarray(b1[sl], dtype=np.float32).reshape(NPAIR, 128).T,
            ],
            axis=1,
        )  # (128, 2, NPAIR)
        in_maps.append(
            {
                "xt": xt,
                "w0c": np.ascontiguousarray(w0ck),
                "w1c": np.ascontiguousarray(w1ck),
                "bp": np.ascontiguousarray(bpc),
            }
        )
    return in_maps


def _postprocess(results):
    outs = []
    for c in range(N_CORES):
        o = results[c]["out"]  # (NPAIR, 128, B) fp16 = [p, q*64+j, b]
        o = (
            o.reshape(NPAIR, 2, 64, B)
            .transpose(3, 0, 1, 2)
            .reshape(B, GPC, D2)
            .astype(np.float32)
        )
        outs.append(o)
    return np.ascontiguousarray(np.concatenate(outs, axis=1))


def _run(inputs, trace=False):
    nc = _build()
    in_maps = _prepare_in_maps(**inputs)
    res = run_bass_kernel_spmd(
        nc, in_maps, core_ids=list(range(N_CORES)), trace=trace
    )
    return _postprocess(res.results), res


def kernel(x, W0, b0, W1, b1):
    out, _ = _run({"x": x, "W0": W0, "b0": b0, "W1": W1, "b1": b1})
    return out


# revision 31
# speedup vs baseline: 1.0082x; 1.0082x over previous
"""Grouped 2-layer MLP (ConvNN) Trainium2 kernel — 4-quadrant PE tiling.

Math (per group g of SIZE=2048):
    h[b,g,:]   = LeakyReLU_0.2(W0[g] @ x[b] + b0[g])     (64 -> 64)
    out[b,g,:] = W1[g] @ h[b,g,:] + b1[g]                (64 -> 64)

Strategy:
  - Shard the group axis over 8 cores (256 groups/core = 128 pairs),
    fully independent, no collectives.
  - Both layers contract over K=64, so the 128x128 PE array is split
    into four independent 64x64 tiles (tile_position mode 64x64:
    T0=(0,0), T2=(0,64), T8=(64,0), T10=(64,64)).  Per slot t all four
    quadrants stream CONCURRENTLY on pair t (layer 0: T8+T2) and pair
    t-LAG (layer 1: T0+T10), each quadrant covering B=1024 in two
    N=512 matmuls with CROSSED banks so the two concurrent streams of
    a layer never sustain writes to the same PSUM bank:
      phase 0: T8 -> hb0[0:64],  T2 -> hb1[64:128]   (and T0/T10 on out)
      phase 1: T8 -> hb1[0:64],  T2 -> hb0[64:128]
    Phase interleaving also keeps a quadrant's two matmuls nonadjacent
    in pc order — matmul STARTS are pc-monotone, so adjacent same-
    quadrant pairs would cascade every later start (measured 1.95us
    slots instead of ~1.24us).  Completing each pair's [128,1024] psum
    tile WITHIN its own slot (not staggered across two) is what lets
    the 2-buffer psum rotation run at P ~= 1.24us: write(slot s) ->
    evac(s..s+1) -> reuse(s+2).
  - x is duplicated on both partition halves so the two layer-0
    quadrants (disjoint SBUF partition ranges) share the xbus without
    conflict.  Stationaries are compact [64,64] slices — no block-
    diagonal zero padding — and each quadrant's second matmul reuses
    the loaded weights (ldweights=False; per-quadrant weight state).
  - Evacuation is the bottleneck: only ACT/DVE can read PSUM, fp32
    psum caps DVE at 1x.  ACT Prelu(+b0, fp16 out) drains h (1.11us
    per [128,1024]); DVE tensor_scalar(+b1, fp16 out) drains out
    (1.285us dur, 1.192us issue rate = (120+1024)/0.96 exactly).  The
    binding loop is obank-reuse: T0(p) waits DVE(p-2); in-slot 1.07us
    to the last out-write + 1.285us DVE -> P ~= 1.2-1.25us.  Splitting
    evacs per-bank measured WORSE (fixed ~220-260ns/op overhead), and
    16-bit PSUM (which would give DVE 2x) is TRN3-only.
  - PE clock: HAM un-throttles to 2.4 GHz ~20us in PROVIDED the
    pipeline never stalls >3.4us (a single mid-kernel stall measured
    re-throttling it to 1.2 GHz for the rest of the run).  Cold
    per-quadrant cycle 854 stream + 203 drain + 125 ldweights ~=
    1.18us just fits under the DVE; warm (~380-400ns/MM) leaves slack,
    so the steady state locks to the DVE at exactly 1192ns/slot (974
    of 1024 MMs measured warm).  Same-bank concurrent PE writes to
    different partition halves are legal (verified correct); only
    sustained same-bank streams are slow.
  - opool bufs=12 is load-bearing: with 5, DVE(p) WARs on store(p-5),
    whose completion is delayed ~8us at startup by bulk-weight DMA
    contention — that one stall cost 4.6us AND re-throttled the PE.
    (bufs=24 regressed 30us — SBUF layout shifts are fragile here;
    so did moving startup loads to the sync ring or merging them.)
  - Output written fp16 as [pair, 128, B] (256KB/pair on the sync
    HWDGE ring; inputs ride gpsimd SWDGE; the per-dma_start ~0.7us Q7
    descriptor-gen cost keeps the startup head to 3 transfers),
    un-transposed/cast to fp32 on the host.

History: full-array baseline 271us (PE-streaming-bound, 512 serial
matmuls) -> 263us (quadrants, naive order) -> 182us (phase
interleave + in-slot psum completion) -> 179us (pool tuning) ->
174us (opool 12 kills the ramp stall; PE stays warm; DVE pegged at
its 1192ns theoretical rate) -> ~173us (LAG=2, 32KB first weight
chunk).  Floor: ~10.5us boot+loads, 128 x 1.192us DVE stream,
~4us tail.  DO NOT perturb the DMA/dependency structure casually:
splitting the x load into 2 transfers, merging head loads into one
tile, sync-ring startup loads, and opool=24 EACH regressed ~30us
(~270ns/slot — consistent with the DVE losing its op-to-op dispatch
overlap when its dependency/sem pattern changes)."""

from contextlib import ExitStack

import numpy as np

import concourse.bass as bass
import concourse.mybir as mybir
import concourse.tile as tile
from concourse.bass_utils import run_bass_kernel_spmd

B = 1024
IN_DIM = 64
SIZE = 2048
D1 = 64
D2 = 64
NEG_SLOPE = 0.2
N_CORES = 8
GPC = SIZE // N_CORES  # 256 groups per core
NPAIR = GPC // 2  # 128 group-pairs per core
LAG = 2  # software-pipeline distance between layer 0 and layer 1

_NC_CACHE = None


def _build():
    global _NC_CACHE
    if _NC_CACHE is not None:
        return _NC_CACHE

    f32 = mybir.dt.float32
    f16 = mybir.dt.float16

    nc = bass.Bass()
    xt = nc.declare_dram_parameter("xt", [128, B], f16, isOutput=False)
    w0c = nc.declare_dram_parameter("w0c", [128, NPAIR, 64], f16, isOutput=False)
    w1c = nc.declare_dram_parameter("w1c", [128, NPAIR, 64], f16, isOutput=False)
    bp = nc.declare_dram_parameter("bp", [128, 2, NPAIR], f32, isOutput=False)
    out = nc.declare_dram_parameter("out", [NPAIR, 128, B], f16, isOutput=True)

    with ExitStack() as ctx:
        tc = ctx.enter_context(tile.TileContext(nc))
        singles = ctx.enter_context(tc.tile_pool(name="singles", bufs=1))
        hpool = ctx.enter_context(tc.tile_pool(name="hpool", bufs=6))
        opool = ctx.enter_context(tc.tile_pool(name="opool", bufs=12))
        pspool = ctx.enter_context(tc.tile_pool(name="psum", bufs=2, space="PSUM"))

        # Startup-critical loads first (each gpsimd dma_start costs
        # ~0.7us of Q7 descriptor-gen, so the head uses as few transfers
        # as possible): xt + w0 head gate T8(0); biases gate ACT(0); w1
        # head gates T0(0) at slot LAG.  Bulk weights stream behind.
        xsb = singles.tile([128, B], f16)
        nc.gpsimd.dma_start(out=xsb, in_=xt[:])
        w0sb = singles.tile([128, NPAIR, 64], f16)
        w1sb = singles.tile([128, NPAIR, 64], f16)
        bsb = singles.tile([128, 2, NPAIR], f32)
        b0sb = bsb[:, 0, :]
        b1sb = bsb[:, 1, :]
        edges = [0, 2, 8, 40, 72, 104, NPAIR]
        nc.gpsimd.dma_start(
            out=w0sb[:, edges[0] : edges[1], :], in_=w0c[:, edges[0] : edges[1], :]
        )
        nc.gpsimd.dma_start(out=bsb, in_=bp[:])
        nc.gpsimd.dma_start(
            out=w1sb[:, edges[0] : edges[1], :], in_=w1c[:, edges[0] : edges[1], :]
        )
        for c in range(1, len(edges) - 1):
            sl = slice(edges[c], edges[c + 1])
            nc.gpsimd.dma_start(out=w0sb[:, sl, :], in_=w0c[:, sl, :])
            nc.gpsimd.dma_start(out=w1sb[:, sl, :], in_=w1c[:, sl, :])

        def w0_at(t):
            return w0sb[:, t, :]

        def w1_at(p):
            return w1sb[:, p, :]

        def quad_mm(psum_tile, prange, lhsT, rhs_tile, rrange, tile_pos, col_half, first):
            """One N=512 matmul (one PSUM bank).  first=False reuses the
            stationary loaded by this quadrant's first matmul of the slot
            (per-tile weight state; other quadrants' LDWEIGHTS don't
            disturb it)."""
            n0, n1 = (0, 512) if col_half == 0 else (512, 1024)
            mm = nc.tensor.matmul(
                psum_tile[prange[0] : prange[1], n0:n1],
                lhsT,
                rhs_tile[rrange[0] : rrange[1], n0:n1],
                start=True,
                stop=True,
                tile_position=tile_pos,
            )
            if not first:
                mm.ins.ldweights = False

        # Dependency-free 1-element Prelu on a private scratch tile:
        # walrus emits the ACT function-table load (~1.3-2.7us) before
        # the FIRST activate of a set, which otherwise sits in the
        # critical ramp chain (slot-0 h -> ACT(0) -> layer 1 -> first
        # DVE).  This hoists it into the boot window.  The scratch tile
        # is touched by nothing else (a w1sb-operand variant of this
        # broke correctness: Tile does not order a later DMA write
        # after an earlier scalar-engine write to the same region).
        actwarm = singles.tile([1, 2], f16)
        nc.scalar.activation(
            out=actwarm[0:1, 1:2],
            in_=actwarm[0:1, 0:1],
            func=mybir.ActivationFunctionType.Prelu,
            bias=0.0,
            scale=1.0,
            alpha=NEG_SLOPE,
        )

        hs_live = {}
        for t in range(NPAIR + LAG):
            p = t - LAG  # pair running layer 1 this slot
            # Both quadrant-pairs of a layer work on the SAME pair within
            # one slot, with CROSSED batch-halves so the two concurrent
            # streams always write different PSUM banks:
            #   phase 0: T8 -> bank0[0:64],  T2 -> bank1[64:128]
            #   phase 1: T8 -> bank1[0:64],  T2 -> bank0[64:128]
            # The pair's full [128,1024] psum tile completes ~1.1us into
            # its own slot, so its evac (issued same slot) frees the
            # 2-buffer rotation a full 2 slots later (P >= ~1.2us), not
            # the ~2.2us the staggered layout forced.  Phases also
            # interleave the quadrants so pc-monotone matmul starts don't
            # cascade (a quadrant's two halves are never adjacent).
            if t < NPAIR:
                hp = pspool.tile([128, B], f32, tag="hps", name=f"hp{t}")
            if 0 <= p < NPAIR:
                op = pspool.tile([128, B], f32, tag="ops", name=f"op{p}")
            for phase in (0, 1):
                first = phase == 0
                if t < NPAIR:  # layer 0 of pair t (T8 + T2)
                    quad_mm(
                        hp, (0, 64), w0_at(t)[64:128, :],
                        xsb, (64, 128), (64, 0), phase, first,
                    )
                    quad_mm(
                        hp, (64, 128), w0_at(t)[0:64, :],
                        xsb, (0, 64), (0, 64), 1 - phase, first,
                    )
                if 0 <= p < NPAIR:  # layer 1 of pair p (T0 + T10)
                    quad_mm(
                        op, (0, 64), w1_at(p)[0:64, :],
                        hs_live[p], (0, 64), (0, 0), phase, first,
                    )
                    quad_mm(
                        op, (64, 128), w1_at(p)[64:128, :],
                        hs_live[p], (64, 128), (64, 64), 1 - phase, first,
                    )
            if t < NPAIR:
                hs = hpool.tile([128, B], f16, tag="h", name=f"hs{t}")
                nc.scalar.activation(
                    out=hs,
                    in_=hp,
                    func=mybir.ActivationFunctionType.Prelu,
                    bias=b0sb[:, t : t + 1],
                    scale=1.0,
                    alpha=NEG_SLOPE,
                )
                hs_live[t] = hs
            if 0 <= p < NPAIR:
                hs_live.pop(p)
                osb = opool.tile([128, B], f16, tag="o", name=f"os{p}")
                nc.vector.tensor_scalar_add(osb, op, b1sb[:, p : p + 1])
                nc.sync.dma_start(out=out[p], in_=osb)

    _dedupe_ldweights(nc)
    _split_multi_waits(nc)
    _NC_CACHE = nc
    return nc


def _dedupe_ldweights(nc):
    """Drop LDWEIGHTS whose AP is identical to the previous one on the PE
    queue (the two half-batch matmuls of a (pair, quadrant) share one
    stationary), migrating semaphore info onto the next PE instruction."""
    import json

    def ldw_key(inst):
        try:
            return mybir.instruction_to_pretty_json_string(inst)
        except Exception:
            return None

    def strip_name(js):
        d = json.loads(js)
        d.pop("name", None)
        d.pop("debug", None)
        d.pop("sync_info", None)
        return json.dumps(d, sort_keys=True)

    n = 0
    for f in nc.m.functions:
        for bb in f.blocks:
            prev_key = None
            out_insts = []
            pending_sync = None
            for inst in bb.instructions:
                eng = getattr(inst, "engine", None)
                if isinstance(inst, mybir.InstLdweights):
                    key = ldw_key(inst)
                    key = strip_name(key) if key else None
                    if key is not None and key == prev_key:
                        si = inst.sync_info
                        if si is not None and (si.on_wait or si.on_update):
                            pending_sync = si
                        n += 1
                        continue  # drop duplicate
                    prev_key = key
                elif eng == mybir.EngineType.PE and pending_sync is not None:
                    si = inst.sync_info
                    waits = list(pending_sync.on_wait or [])
                    upds = list(pending_sync.on_update or [])
                    if si is not None:
                        waits += list(si.on_wait or [])
                        upds += list(si.on_update or [])
                    inst.sync_info = mybir.SyncInfo(on_wait=waits, on_update=upds)
                    pending_sync = None
                out_insts.append(inst)
            assert pending_sync is None, "dangling sync from dropped ldweights"
            bb.instructions = out_insts
    return n


def _split_multi_waits(nc):
    """Walrus allows at most ONE semaphore wait per instruction; hoist
    extras onto same-engine NoOp carriers directly before it."""
    import bass_rust

    n = 0
    for f in nc.m.functions:
        for bb in f.blocks:
            out_insts = []
            changed = False
            for inst in bb.instructions:
                si = inst.sync_info
                waits = list(si.on_wait) if si is not None and si.on_wait else []
                if len(waits) > 1:
                    changed = True
                    for w in waits[:-1]:
                        nop = bass_rust.InstNoOp(
                            name=f"{inst.name}-sw{n}", engine=inst.engine
                        )
                        n += 1
                        nop.sync_info = mybir.SyncInfo(on_wait=[w], on_update=[])
                        out_insts.append(nop)
                    inst.sync_info = mybir.SyncInfo(
                        on_wait=[waits[-1]],
                        on_update=list(si.on_update) if si.on_update else [],
                    )
                out_insts.append(inst)
            if changed:
                bb.instructions = out_insts
    return nc


def _prepare_in_maps(x, W0, b0, W1, b1):
    x = np.asarray(x, dtype=np.float32)
    xt1 = np.ascontiguousarray(x.T).astype(np.float16)  # (64, 1024)
    xt = np.concatenate([xt1, xt1], axis=0)  # (128, 1024) duplicated halves
    in_maps = []
    for c in range(N_CORES):
        sl = slice(c * GPC, (c + 1) * GPC)
        W0c = np.asarray(W0[sl], dtype=np.float32).reshape(NPAIR, 2, D1, IN_DIM)
        W1c = np.asarray(W1[sl], dtype=np.float32).reshape(NPAIR, 2, D2, D1)
        # w0[64+k, p, j] = W0[2p, j, k] (A, tile T8 rows 64-127)
        # w0[k,    p, j] = W0[2p+1, j, k] (B, tile T2 rows 0-63)
        w0ck = np.empty((128, NPAIR, 64), dtype=np.float16)
        w0ck[64:128] = W0c[:, 0].transpose(2, 0, 1)
        w0ck[0:64] = W0c[:, 1].transpose(2, 0, 1)
        # w1[k,    p, j] = W1[2p, j, k] (A, tile T0 rows 0-63)
        # w1[64+k, p, j] = W1[2p+1, j, k] (B, tile T10 rows 64-127)
        w1ck = np.empty((128, NPAIR, 64), dtype=np.float16)
        w1ck[0:64] = W1c[:, 0].transpose(2, 0, 1)
        w1ck[64:128] = W1c[:, 1].transpose(2, 0, 1)

        bpc = np.stack(
            [
                np.asarray(b0[sl], dtype=np.float32).reshape(NPAIR, 128).T,
                np.asarray(b1[sl], dtype=np.float32).reshape(NPAIR, 128).T,
            ],
            axis=1,
        )  # (128, 2, NPAIR)
        in_maps.append(
            {
                "xt": xt,
                "w0c": np.ascontiguousarray(w0ck),
                "w1c": np.ascontiguousarray(w1ck),
                "bp": np.ascontiguousarray(bpc),
            }
        )
    return in_maps


def _postprocess(results):
    outs = []
    for c in range(N_CORES):
        o = results[c]["out"]  # (NPAIR, 128, B) fp16 = [p, q*64+j, b]
        o = (
            o.reshape(NPAIR, 2, 64, B)
            .transpose(3, 0, 1, 2)
            .reshape(B, GPC, D2)
            .astype(np.float32)
        )
        outs.append(o)
    return np.ascontiguousarray(np.concatenate(outs, axis=1))


def _run(inputs, trace=False):
    nc = _build()
    in_maps = _prepare_in_maps(**inputs)
    res = run_bass_kernel_spmd(
        nc, in_maps, core_ids=list(range(N_CORES)), trace=trace
    )
    return _postprocess(res.results), res


def kernel(x, W0, b0, W1, b1):
    out, _ = _run({"x": x, "W0": W0, "b0": b0, "W1": W1, "b1": b1})
    return out


# revision 32
# speedup vs baseline: 1.0108x; 1.0026x over previous
"""Grouped 2-layer MLP (ConvNN) Trainium2 kernel — 4-quadrant PE tiling.

Math (per group g of SIZE=2048):
    h[b,g,:]   = LeakyReLU_0.2(W0[g] @ x[b] + b0[g])     (64 -> 64)
    out[b,g,:] = W1[g] @ h[b,g,:] + b1[g]                (64 -> 64)

Strategy:
  - Shard the group axis over 8 cores (256 groups/core = 128 pairs),
    fully independent, no collectives.
  - Both layers contract over K=64, so the 128x128 PE array is split
    into four independent 64x64 tiles (tile_position mode 64x64:
    T0=(0,0), T2=(0,64), T8=(64,0), T10=(64,64)).  Per slot t all four
    quadrants stream CONCURRENTLY on pair t (layer 0: T8+T2) and pair
    t-LAG (layer 1: T0+T10), each quadrant covering B=1024 in two
    N=512 matmuls with CROSSED banks so the two concurrent streams of
    a layer never sustain writes to the same PSUM bank:
      phase 0: T8 -> hb0[0:64],  T2 -> hb1[64:128]   (and T0/T10 on out)
      phase 1: T8 -> hb1[0:64],  T2 -> hb0[64:128]
    Phase interleaving also keeps a quadrant's two matmuls nonadjacent
    in pc order — matmul STARTS are pc-monotone, so adjacent same-
    quadrant pairs would cascade every later start (measured 1.95us
    slots instead of ~1.24us).  Completing each pair's [128,1024] psum
    tile WITHIN its own slot (not staggered across two) is what lets
    the 2-buffer psum rotation run at P ~= 1.24us: write(slot s) ->
    evac(s..s+1) -> reuse(s+2).
  - x is duplicated on both partition halves so the two layer-0
    quadrants (disjoint SBUF partition ranges) share the xbus without
    conflict.  Stationaries are compact [64,64] slices — no block-
    diagonal zero padding — and each quadrant's second matmul reuses
    the loaded weights (ldweights=False; per-quadrant weight state).
  - Evacuation is the bottleneck: only ACT/DVE can read PSUM, fp32
    psum caps DVE at 1x.  ACT Prelu(+b0, fp16 out) drains h (1.11us
    per [128,1024]); DVE tensor_scalar(+b1, fp16 out) drains out
    (1.285us dur, 1.192us issue rate = (120+1024)/0.96 exactly).  The
    binding loop is obank-reuse: T0(p) waits DVE(p-2); in-slot 1.07us
    to the last out-write + 1.285us DVE -> P ~= 1.2-1.25us.  Splitting
    evacs per-bank measured WORSE (fixed ~220-260ns/op overhead), and
    16-bit PSUM (which would give DVE 2x) is TRN3-only.
  - PE clock: HAM un-throttles to 2.4 GHz ~20us in PROVIDED the
    pipeline never stalls >3.4us (a single mid-kernel stall measured
    re-throttling it to 1.2 GHz for the rest of the run).  Cold
    per-quadrant cycle 854 stream + 203 drain + 125 ldweights ~=
    1.18us just fits under the DVE; warm (~380-400ns/MM) leaves slack,
    so the steady state locks to the DVE at exactly 1192ns/slot (974
    of 1024 MMs measured warm).  Same-bank concurrent PE writes to
    different partition halves are legal (verified correct); only
    sustained same-bank streams are slow.
  - opool bufs=12 is load-bearing: with 5, DVE(p) WARs on store(p-5),
    whose completion is delayed ~8us at startup by bulk-weight DMA
    contention — that one stall cost 4.6us AND re-throttled the PE.
    (bufs=24 regressed 30us — SBUF layout shifts are fragile here;
    so did moving startup loads to the sync ring or merging them.)
  - Output written fp16 as [pair, 128, B] (256KB/pair on the sync
    HWDGE ring; inputs ride gpsimd SWDGE; the per-dma_start ~0.7us Q7
    descriptor-gen cost keeps the startup head to 3 transfers),
    un-transposed/cast to fp32 on the host.

History: full-array baseline 271us (PE-streaming-bound, 512 serial
matmuls) -> 263us (quadrants, naive order) -> 182us (phase
interleave + in-slot psum completion) -> 179us (pool tuning) ->
174us (opool 12 kills the ramp stall; PE stays warm; DVE pegged at
its 1192ns theoretical rate) -> ~173us (LAG=2, 32KB first weight
chunk) -> ~172.1us (ACT function-table load hoisted into the boot
window via a dependency-free 1-element Prelu on a private scratch
tile).  Floor: ~9.5us boot+loads, 128 x 1.192us DVE stream, ~4us
tail.  DO NOT perturb the DMA/dependency structure casually:
splitting the x load into 2 transfers, merging head loads into one
tile, sync-ring startup loads, and opool=24 EACH regressed ~30us
(~270ns/slot — consistent with the DVE losing its op-to-op dispatch
overlap when its dependency/sem pattern changes)."""

from contextlib import ExitStack

import numpy as np

import concourse.bass as bass
import concourse.mybir as mybir
import concourse.tile as tile
from concourse.bass_utils import run_bass_kernel_spmd

B = 1024
IN_DIM = 64
SIZE = 2048
D1 = 64
D2 = 64
NEG_SLOPE = 0.2
N_CORES = 8
GPC = SIZE // N_CORES  # 256 groups per core
NPAIR = GPC // 2  # 128 group-pairs per core
LAG = 2  # software-pipeline distance between layer 0 and layer 1

_NC_CACHE = None


def _build():
    global _NC_CACHE
    if _NC_CACHE is not None:
        return _NC_CACHE

    f32 = mybir.dt.float32
    f16 = mybir.dt.float16

    nc = bass.Bass()
    xt = nc.declare_dram_parameter("xt", [128, B], f16, isOutput=False)
    w0c = nc.declare_dram_parameter("w0c", [128, NPAIR, 64], f16, isOutput=False)
    w1c = nc.declare_dram_parameter("w1c", [128, NPAIR, 64], f16, isOutput=False)
    bp = nc.declare_dram_parameter("bp", [128, 2, NPAIR], f32, isOutput=False)
    out = nc.declare_dram_parameter("out", [NPAIR, 128, B], f16, isOutput=True)

    with ExitStack() as ctx:
        tc = ctx.enter_context(tile.TileContext(nc))
        singles = ctx.enter_context(tc.tile_pool(name="singles", bufs=1))
        hpool = ctx.enter_context(tc.tile_pool(name="hpool", bufs=6))
        opool = ctx.enter_context(tc.tile_pool(name="opool", bufs=12))
        pspool = ctx.enter_context(tc.tile_pool(name="psum", bufs=2, space="PSUM"))

        # Startup-critical loads first (each gpsimd dma_start costs
        # ~0.7us of Q7 descriptor-gen, so the head uses as few transfers
        # as possible): xt + w0 head gate T8(0); biases gate ACT(0); w1
        # head gates T0(0) at slot LAG.  Bulk weights stream behind.
        xsb = singles.tile([128, B], f16)
        nc.gpsimd.dma_start(out=xsb, in_=xt[:])
        w0sb = singles.tile([128, NPAIR, 64], f16)
        w1sb = singles.tile([128, NPAIR, 64], f16)
        bsb = singles.tile([128, 2, NPAIR], f32)
        b0sb = bsb[:, 0, :]
        b1sb = bsb[:, 1, :]
        edges = [0, 2, 8, 40, 72, 104, NPAIR]
        nc.gpsimd.dma_start(
            out=w0sb[:, edges[0] : edges[1], :], in_=w0c[:, edges[0] : edges[1], :]
        )
        nc.gpsimd.dma_start(out=bsb, in_=bp[:])
        nc.gpsimd.dma_start(
            out=w1sb[:, edges[0] : edges[1], :], in_=w1c[:, edges[0] : edges[1], :]
        )
        for c in range(1, len(edges) - 1):
            sl = slice(edges[c], edges[c + 1])
            nc.gpsimd.dma_start(out=w0sb[:, sl, :], in_=w0c[:, sl, :])
            nc.gpsimd.dma_start(out=w1sb[:, sl, :], in_=w1c[:, sl, :])

        def w0_at(t):
            return w0sb[:, t, :]

        def w1_at(p):
            return w1sb[:, p, :]

        def quad_mm(psum_tile, prange, lhsT, rhs_tile, rrange, tile_pos, col_half, first):
            """One N=512 matmul (one PSUM bank).  first=False reuses the
            stationary loaded by this quadrant's first matmul of the slot
            (per-tile weight state; other quadrants' LDWEIGHTS don't
            disturb it)."""
            n0, n1 = (0, 512) if col_half == 0 else (512, 1024)
            mm = nc.tensor.matmul(
                psum_tile[prange[0] : prange[1], n0:n1],
                lhsT,
                rhs_tile[rrange[0] : rrange[1], n0:n1],
                start=True,
                stop=True,
                tile_position=tile_pos,
            )
            if not first:
                mm.ins.ldweights = False

        # Dependency-free 1-element Prelu on a private scratch tile:
        # walrus emits the ACT function-table load (~1.3-2.7us) before
        # the FIRST activate of a set, which otherwise sits in the
        # critical ramp chain (slot-0 h -> ACT(0) -> layer 1 -> first
        # DVE).  This hoists it into the boot window.  The scratch tile
        # is touched by nothing else (a w1sb-operand variant of this
        # broke correctness: Tile does not order a later DMA write
        # after an earlier scalar-engine write to the same region).
        actwarm = singles.tile([1, 2], f16)
        nc.scalar.activation(
            out=actwarm[0:1, 1:2],
            in_=actwarm[0:1, 0:1],
            func=mybir.ActivationFunctionType.Prelu,
            bias=0.0,
            scale=1.0,
            alpha=NEG_SLOPE,
        )

        hs_live = {}
        for t in range(NPAIR + LAG):
            p = t - LAG  # pair running layer 1 this slot
            # Both quadrant-pairs of a layer work on the SAME pair within
            # one slot, with CROSSED batch-halves so the two concurrent
            # streams always write different PSUM banks:
            #   phase 0: T8 -> bank0[0:64],  T2 -> bank1[64:128]
            #   phase 1: T8 -> bank1[0:64],  T2 -> bank0[64:128]
            # The pair's full [128,1024] psum tile completes ~1.1us into
            # its own slot, so its evac (issued same slot) frees the
            # 2-buffer rotation a full 2 slots later (P >= ~1.2us), not
            # the ~2.2us the staggered layout forced.  Phases also
            # interleave the quadrants so pc-monotone matmul starts don't
            # cascade (a quadrant's two halves are never adjacent).
            if t < NPAIR:
                hp = pspool.tile([128, B], f32, tag="hps", name=f"hp{t}")
            if 0 <= p < NPAIR:
                op = pspool.tile([128, B], f32, tag="ops", name=f"op{p}")
            for phase in (0, 1):
                first = phase == 0
                if t < NPAIR:  # layer 0 of pair t (T8 + T2)
                    quad_mm(
                        hp, (0, 64), w0_at(t)[64:128, :],
                        xsb, (64, 128), (64, 0), phase, first,
                    )
                    quad_mm(
                        hp, (64, 128), w0_at(t)[0:64, :],
                        xsb, (0, 64), (0, 64), 1 - phase, first,
                    )
                if 0 <= p < NPAIR:  # layer 1 of pair p (T0 + T10)
                    quad_mm(
                        op, (0, 64), w1_at(p)[0:64, :],
                        hs_live[p], (0, 64), (0, 0), phase, first,
                    )
                    quad_mm(
                        op, (64, 128), w1_at(p)[64:128, :],
                        hs_live[p], (64, 128), (64, 64), 1 - phase, first,
                    )
            if t < NPAIR:
                hs = hpool.tile([128, B], f16, tag="h", name=f"hs{t}")
                nc.scalar.activation(
                    out=hs,
                    in_=hp,
                    func=mybir.ActivationFunctionType.Prelu,
                    bias=b0sb[:, t : t + 1],
                    scale=1.0,
                    alpha=NEG_SLOPE,
                )
                hs_live[t] = hs
            if 0 <= p < NPAIR:
                hs_live.pop(p)
                osb = opool.tile([128, B], f16, tag="o", name=f"os{p}")
                nc.vector.tensor_scalar_add(osb, op, b1sb[:, p : p + 1])
                nc.sync.dma_start(out=out[p], in_=osb)

    _dedupe_ldweights(nc)
    _split_multi_waits(nc)
    _NC_CACHE = nc
    return nc


def _dedupe_ldweights(nc):
    """Drop LDWEIGHTS whose AP is identical to the previous one on the PE
    queue (the two half-batch matmuls of a (pair, quadrant) share one
    stationary), migrating semaphore info onto the next PE instruction."""
    import json

    def ldw_key(inst):
        try:
            return mybir.instruction_to_pretty_json_string(inst)
        except Exception:
            return None

    def strip_name(js):
        d = json.loads(js)
        d.pop("name", None)
        d.pop("debug", None)
        d.pop("sync_info", None)
        return json.dumps(d, sort_keys=True)

    n = 0
    for f in nc.m.functions:
        for bb in f.blocks:
            prev_key = None
            out_insts = []
            pending_sync = None
            for inst in bb.instructions:
                eng = getattr(inst, "engine", None)
                if isinstance(inst, mybir.InstLdweights):
                    key = ldw_key(inst)
                    key = strip_name(key) if key else None
                    if key is not None and key == prev_key:
                        si = inst.sync_info
                        if si is not None and (si.on_wait or si.on_update):
                            pending_sync = si
                        n += 1
                        continue  # drop duplicate
                    prev_key = key
                elif eng == mybir.EngineType.PE and pending_sync is not None:
                    si = inst.sync_info
                    waits = list(pending_sync.on_wait or [])
                    upds = list(pending_sync.on_update or [])
                    if si is not None:
                        waits += list(si.on_wait or [])
                        upds += list(si.on_update or [])
                    inst.sync_info = mybir.SyncInfo(on_wait=waits, on_update=upds)
                    pending_sync = None
                out_insts.append(inst)
            assert pending_sync is None, "dangling sync from dropped ldweights"
            bb.instructions = out_insts
    return n


def _split_multi_waits(nc):
    """Walrus allows at most ONE semaphore wait per instruction; hoist
    extras onto same-engine NoOp carriers directly before it."""
    import bass_rust

    n = 0
    for f in nc.m.functions:
        for bb in f.blocks:
            out_insts = []
            changed = False
            for inst in bb.instructions:
                si = inst.sync_info
                waits = list(si.on_wait) if si is not None and si.on_wait else []
                if len(waits) > 1:
                    changed = True
                    for w in waits[:-1]:
                        nop = bass_rust.InstNoOp(
                            name=f"{inst.name}-sw{n}", engine=inst.engine
                        )
                        n += 1
                        nop.sync_info = mybir.SyncInfo(on_wait=[w], on_update=[])
                        out_insts.append(nop)
                    inst.sync_info = mybir.SyncInfo(
                        on_wait=[waits[-1]],
                        on_update=list(si.on_update) if si.on_update else [],
                    )
                out_insts.append(inst)
            if changed:
                bb.instructions = out_insts
    return nc


def _prepare_in_maps(x, W0, b0, W1, b1):
    x = np.asarray(x, dtype=np.float32)
    xt1 = np.ascontiguousarray(x.T).astype(np.float16)  # (64, 1024)
    xt = np.concatenate([xt1, xt1], axis=0)  # (128, 1024) duplicated halves
    in_maps = []
    for c in range(N_CORES):
        sl = slice(c * GPC, (c + 1) * GPC)
        W0c = np.asarray(W0[sl], dtype=np.float32).reshape(NPAIR, 2, D1, IN_DIM)
        W1c = np.asarray(W1[sl], dtype=np.float32).reshape(NPAIR, 2, D2, D1)
        # w0[64+k, p, j] = W0[2p, j, k] (A, tile T8 rows 64-127)
        # w0[k,    p, j] = W0[2p+1, j, k] (B, tile T2 rows 0-63)
        w0ck = np.empty((128, NPAIR, 64), dtype=np.float16)
        w0ck[64:128] = W0c[:, 0].transpose(2, 0, 1)
        w0ck[0:64] = W0c[:, 1].transpose(2, 0, 1)
        # w1[k,    p, j] = W1[2p, j, k] (A, tile T0 rows 0-63)
        # w1[64+k, p, j] = W1[2p+1, j, k] (B, tile T10 rows 64-127)
        w1ck = np.empty((128, NPAIR, 64), dtype=np.float16)
        w1ck[0:64] = W1c[:, 0].transpose(2, 0, 1)
        w1ck[64:128] = W1c[:, 1].transpose(2, 0, 1)

        bpc = np.stack(
            [
                np.asarray(b0[sl], dtype=np.float32).reshape(NPAIR, 128).T,
                np.asarray(b1[sl], dtype=np.float32).reshape(NPAIR, 128).T,
            ],
            axis=1,
        )  # (128, 2, NPAIR)
        in_maps.append(
            {
                "xt": xt,
                "w0c": np.ascontiguousarray(w0ck),
                "w1c": np.ascontiguousarray(w1ck),
                "bp": np.ascontiguousarray(bpc),
            }
        )
    return in_maps


def _postprocess(results):
    outs = []
    for c in range(N_CORES):
        o = results[c]["out"]  # (NPAIR, 128, B) fp16 = [p, q*64+j, b]
        o = (
            o.reshape(NPAIR, 2, 64, B)
            .transpose(3, 0, 1, 2)
            .reshape(B, GPC, D2)
            .astype(np.float32)
        )
        outs.append(o)
    return np.ascontiguousarray(np.concatenate(outs, axis=1))


def _run(inputs, trace=False):
    nc = _build()
    in_maps = _prepare_in_maps(**inputs)
    res = run_bass_kernel_spmd(
        nc, in_maps, core_ids=list(range(N_CORES)), trace=trace
    )
    return _postprocess(res.results), res


def kernel(x, W0, b0, W1, b1):
    out, _ = _run({"x": x, "W0": W0, "b0": b0, "W1": W1, "b1": b1})
    return out
